# revision 12
# baseline (speedup 1.0000x reference)
"""Multi-head attention (RoPE, causal) Bass kernel for 8 TRN2 NeuronCores.

Sharding: 2-way batch x 4-way heads (4 heads per core).

Schedule (v4): software-pipelined at the instruction level so the PE never
idles (idling >3.4us re-throttles the PE clock to 1.2 GHz):
  - prologue: projection of i-slice 0, pair-0 first; pair-1 projections are
    queued as stage-0 filler
  - stage it: attention over slice it; projection MMs of slice it+1 and
    out-projection MMs of slice it-1 interleave into the attention stream
    as PE filler, paced by estimated duration
  - attention issues score-MMs of chunk-pair g+1 BEFORE ctx-MMs of pair g
    so the exp (ACT) latency of pair g hides behind PE work
Mechanics:
  - bf16 data path (DMA halved, FWL weight loads); PSUM stays f32
  - causal mask folded into score PSUM via one identity-matmul bias per
    diagonal chunk pair (-30k upper triangle, skewed output AP)
  - softmax 1/l via reciprocal_approx_fast off an SBUF-staged l row;
    denominator comes free from a ones-column appended to V
  - norm of head h deferred to head h+1; out-proj of slice it deferred
    into stage it+1 as filler
  - input DMAs dispatched across engine queues in need-order; epilogue
    output DMAs fan out over four queues
"""
import numpy as np
from contextlib import ExitStack

import ml_dtypes
import concourse.bass as bass
import concourse.tile as tile
from concourse import bacc, mybir
from concourse.bass_utils import run_bass_kernel_spmd

D_IN = 1024
D_OUT = 1024
HD = 64                   # head dim
S = 2048                  # sequence length
B = 2
THETA = 10000.0
NCORES = 8
IS = 512                  # i-slice width
NIS = S // IS             # 4 i-slices
NJC = S // 128            # 16 j-chunks
NEG = -30000.0            # causal mask bias (exp(NEG/8) == 0 in f32)

F32 = mybir.dt.float32
BF16 = mybir.dt.bfloat16
BFNP = ml_dtypes.bfloat16


class Fill:
    """Filler queue: PE work items popped into attention group gaps,
    paced so that `done/total` tracks the attention progress fraction."""

    def __init__(self):
        self.items = []
        self.total = 1e-9
        self.done = 0.0
        self.markers = {}
        self._next_marker = 0

    def add(self, est, fn):
        self.items.append((est, fn))
        self.total += est

    def add_marker(self):
        mid = self._next_marker
        self._next_marker += 1
        self.markers[mid] = len(self.items)
        return mid

    def pop(self, frac):
        target = frac * self.total
        while self.items and self.done < target:
            self._pop_one()

    def pop_marker(self, mid):
        while self.markers.get(mid, 0) > 0 and self.items:
            self._pop_one()

    def _pop_one(self):
        est, fn = self.items.pop(0)
        fn()
        self.done += est
        for k in self.markers:
            if self.markers[k] > 0:
                self.markers[k] -= 1

    def drain(self):
        while self.items:
            self._pop_one()


def build_kernel():
    nc = bacc.Bacc("TRN2", target_bir_lowering=False, debug=False)

    # host pre-shuffled so every DMA is contiguous per partition:
    # xtr[it, p, c, i] = x[b]^T[128c+p, 512it+i]; w*r[p, c, n] = W[128c+p, n]
    xT = nc.dram_tensor("xT", [NIS, 128, 8, IS], BF16, kind="ExternalInput").ap()
    wq = nc.dram_tensor("wq", [128, 8, 256], BF16, kind="ExternalInput").ap()
    wk = nc.dram_tensor("wk", [128, 8, 256], BF16, kind="ExternalInput").ap()
    wv = nc.dram_tensor("wv", [128, 8, 256], BF16, kind="ExternalInput").ap()
    wo = nc.dram_tensor("wo", [128, 2, 1024], BF16, kind="ExternalInput").ap()
    cdup = nc.dram_tensor("cdup", [128, S], BF16, kind="ExternalInput").ap()
    sdup = nc.dram_tensor("sdup", [128, S], BF16, kind="ExternalInput").ap()
    p64 = nc.dram_tensor("p64", [128, 128], BF16, kind="ExternalInput").ap()
    ident = nc.dram_tensor("ident", [128, 128], BF16, kind="ExternalInput").ap()
    # [tri | tri]: one bias matmul covers both chunks of a diagonal pair
    tneg2 = nc.dram_tensor("tneg2", [128, 256], BF16, kind="ExternalInput").ap()
    onesc = nc.dram_tensor("onesc", [128, 65], BF16, kind="ExternalInput").ap()
    out = nc.dram_tensor("out", [S, D_OUT], BF16, kind="ExternalOutput").ap()

    with tile.TileContext(nc) as tc, ExitStack() as ctx:
        singles = ctx.enter_context(tc.tile_pool(name="singles", bufs=1))
        xpool = ctx.enter_context(tc.tile_pool(name="xpool", bufs=2))
        rope_tmp = ctx.enter_context(tc.tile_pool(name="rope_tmp", bufs=3))
        expp = ctx.enter_context(tc.tile_pool(name="expp", bufs=4))
        bcp = ctx.enter_context(tc.tile_pool(name="bcp", bufs=2))
        ctxp = ctx.enter_context(tc.tile_pool(name="ctxp", bufs=6))
        outp = ctx.enter_context(tc.tile_pool(name="outp", bufs=3))
        # PSUM: quad 2x2 banks + ctx 2x1 + shared proj/perm/vps/ops 2x1 = 8
        ps = ctx.enter_context(tc.tile_pool(name="ps", bufs=2, space="PSUM"))

        # ---- input DMAs: need-ordered, dispatch spread across engines ----
        engs = [nc.sync, nc.scalar, nc.gpsimd]
        wq_t = singles.tile([128, 8, 256], BF16, tag="wq", name="wq")
        xt0_t = xpool.tile([128, 8, IS], BF16, tag="xt", name="xt0")
        for c in range(8):
            e = engs[c % 3]
            e.dma_start(out=wq_t[:, c, :], in_=wq[:, c, :])
            e.dma_start(out=xt0_t[:, c, :], in_=xT[0, :, c, :])
        xts = {0: xt0_t}
        # mid-prologue needs: wk (k-proj), cdup (t1), p64 (perm)
        wk_t = singles.tile([128, 8, 256], BF16, tag="wk", name="wk")
        nc.gpsimd.dma_start(out=wk_t[:, 0:4, :], in_=wk[:, 0:4, :])
        c_sb = singles.tile([128, S], BF16, tag="cdup")
        nc.scalar.dma_start(out=c_sb, in_=cdup)
        p64_sb = singles.tile([128, 128], BF16, tag="p64")
        nc.gpsimd.dma_start(out=p64_sb, in_=p64)
        s_sb = singles.tile([128, S], BF16, tag="sdup")
        nc.sync.dma_start(out=s_sb, in_=sdup)
        nc.gpsimd.dma_start(out=wk_t[:, 4:8, :], in_=wk[:, 4:8, :])
        wv_t = singles.tile([128, 8, 256], BF16, tag="wv", name="wv")
        nc.scalar.dma_start(out=wv_t, in_=wv)

        def xt_dma(it):
            t = xpool.tile([128, 8, IS], BF16, tag="xt", name=f"xt{it}")
            nc.sync.dma_start(out=t, in_=xT[it])
            xts[it] = t

        xt_dma(1)
        onesc_sb = singles.tile([128, 65], BF16, tag="ones")
        nc.scalar.dma_start(out=onesc_sb, in_=onesc)
        ident_sb = singles.tile([128, 128], BF16, tag="ident")
        nc.gpsimd.dma_start(out=ident_sb, in_=ident)
        tneg2_sb = singles.tile([128, 256], BF16, tag="tneg2")
        nc.gpsimd.dma_start(out=tneg2_sb, in_=tneg2)
        wo_sb = singles.tile([128, 2, 1024], BF16, tag="wo")
        nc.scalar.dma_start(out=wo_sb, in_=wo)
        w_sb = {"wq": wq_t, "wk": wk_t, "wv": wv_t}

        # persistent SBUF state
        qt = [singles.tile([128, S], BF16, tag=f"qt{p}", name=f"qt{p}")
              for p in range(2)]
        kt = [singles.tile([128, S], BF16, tag=f"kt{p}", name=f"kt{p}")
              for p in range(2)]
        v4 = singles.tile([128, NJC, 4, 65], BF16, tag="v4")
        v4_ones = bass.AP(tensor=v4.tensor, offset=64,
                          ap=[[NJC * 4 * 65, 128], [65, NJC * 4]])
        nc.vector.tensor_copy(v4_ones, onesc_sb[:, 0:64])
        # 1/l rows at partition 0: [head, it parity, i]
        rl = singles.tile([128, 4, 2, IS], F32, tag="rl")

        # ---------------- projection work-item builders --------------
        def qk_items(jt, tname, p, st, fill):
            wt = w_sb["w" + tname]
            dests = qt if tname == "q" else kt

            def mk(c):
                def fn():
                    if c == 0:
                        st[(tname, p)] = ps.tile(
                            [128, IS], F32, tag="pp", name=f"proj_{tname}{p}")
                    proj = st[(tname, p)]
                    nc.tensor.matmul(proj, wt[:, c, 128 * p:128 * (p + 1)],
                                     xts[jt][:, c, :],
                                     start=(c == 0), stop=(c == 7))
                    if c == 7:
                        raw = rope_tmp.tile([128, IS], BF16, tag="raw")
                        nc.vector.tensor_copy(raw, proj)
                        t1 = rope_tmp.tile([128, IS], BF16, tag="t1")
                        nc.vector.tensor_mul(
                            t1, proj, c_sb[:, jt * IS:(jt + 1) * IS])
                        st[(tname, p, "raw")] = raw
                        st[(tname, p, "t1")] = t1
                return (213, fn)

            for c in range(8):
                fill.add(*mk(c))

        def perm_item(jt, tname, p, st, fill):
            dests = qt if tname == "q" else kt

            def fn():
                perm = ps.tile([128, IS], F32, tag="pp", name="perm")
                nc.tensor.matmul(perm, p64_sb, st[(tname, p, "raw")],
                                 start=True, stop=True)
                t2 = rope_tmp.tile([128, IS], BF16, tag="t2")
                nc.vector.tensor_mul(t2, perm, s_sb[:, jt * IS:(jt + 1) * IS])
                nc.vector.tensor_add(
                    dests[p][:, jt * IS:(jt + 1) * IS],
                    st[(tname, p, "t1")], t2)
            fill.add(230, fn)

        def v_items(jt, half, st, fill):
            def mk(js, c):
                def fn():
                    if js == 0 and c == 0:
                        st[("v", half)] = ps.tile(
                            [128, 2, 256], F32, tag="pp", name=f"vps{half}")
                    vps = st[("v", half)]
                    col = 128 * (half * 2 + js)
                    nc.tensor.matmul(vps[:, js, :],
                                     xts[jt][:, c, col:col + 128],
                                     w_sb["wv"][:, c, :],
                                     start=(c == 0), stop=(c == 7))
                    if c == 7:
                        jtile = jt * 4 + half * 2 + js
                        nc.vector.tensor_copy(
                            v4[:, jtile, :, 0:64],
                            vps[:, js, :].rearrange("p (h d) -> p h d", h=4))
                return (120, fn)

            for js in range(2):
                for c in range(8):
                    fill.add(*mk(js, c))

        def proj_items(jt, fill):
            st = {}
            qk_items(jt, "q", 0, st, fill)
            qk_items(jt, "q", 1, st, fill)
            perm_item(jt, "q", 0, st, fill)
            qk_items(jt, "k", 0, st, fill)
            perm_item(jt, "q", 1, st, fill)
            qk_items(jt, "k", 1, st, fill)
            perm_item(jt, "k", 0, st, fill)
            v_items(jt, 0, st, fill)
            perm_item(jt, "k", 1, st, fill)
            v_items(jt, 1, st, fill)

        # ---------------- out-projection items for slice it ---------------
        def outproj_items(ctxs_pair, it, fill):
            st = {}

            def op_item(ib, nt):
                def fn():
                    if nt == 0:
                        st[ib] = outp.tile([128, 1024], BF16, tag="o",
                                           name="ot")
                    ot = st[ib]
                    ops = ps.tile([128, IS], F32, tag="pp", name="ops")
                    for pair in range(2):
                        nc.tensor.matmul(
                            ops,
                            ctxs_pair[pair][:, 128 * ib:128 * (ib + 1)],
                            wo_sb[:, pair, nt * IS:(nt + 1) * IS],
                            start=(pair == 0), stop=(pair == 1))
                    nc.vector.tensor_copy(ot[:, nt * IS:(nt + 1) * IS], ops)
                    if nt == 1:
                        nc.sync.dma_start(
                            out=out[it * IS + 128 * ib:
                                    it * IS + 128 * (ib + 1), :],
                            in_=ot)
                return (550, fn)

            for ib in range(4):
                for nt in range(2):
                    fill.add(*op_item(ib, nt))

        def outproj_epilogue(ctxs_pair, it):
            # attention is over: quad/ctx PSUM rings are free. Even ib uses
            # a 2-bank quad tile with one N=1024 matmul per pair; odd ib
            # uses two ctx-ring banks. Copies alternate DVE/ACT; output
            # DMAs fan out over the three dispatch queues.
            for ib in range(4):
                ot = outp.tile([128, 1024], BF16, tag="o", name="ot")
                if ib % 2 == 0:
                    ops2 = ps.tile([128, 2, IS], F32, tag="quad", name="ops2")
                    for nt in range(2):
                        for pair in range(2):
                            nc.tensor.matmul(
                                ops2[:, nt, :],
                                ctxs_pair[pair][:, 128 * ib:128 * (ib + 1)],
                                wo_sb[:, pair, nt * IS:(nt + 1) * IS],
                                start=(pair == 0), stop=(pair == 1))
                    halves = (ops2[:, 0, :], ops2[:, 1, :])
                else:
                    o0 = ps.tile([128, IS], F32, tag="ctx", name="ops2a")
                    o1 = ps.tile([128, IS], F32, tag="ctx", name="ops2b")
                    for nt, o in enumerate((o0, o1)):
                        for pair in range(2):
                            nc.tensor.matmul(
                                o, ctxs_pair[pair][:, 128 * ib:128 * (ib + 1)],
                                wo_sb[:, pair, nt * IS:(nt + 1) * IS],
                                start=(pair == 0), stop=(pair == 1))
                    halves = (o0, o1)
                nc.vector.tensor_copy(ot[:, 0:IS], halves[0])
                nc.scalar.copy(ot[:, IS:1024], halves[1])
                engs[ib % 3].dma_start(
                    out=out[it * IS + 128 * ib: it * IS + 128 * (ib + 1), :],
                    in_=ot)

        # ---------------- attention -----------------
        def recip_l(ctx_ps, h, it):
            # custom-DVE reciprocal needs an SBUF input (bitwise seed
            # misreads PSUM): stage the l row through SBUF first
            ls = rope_tmp.tile([1, IS], F32, tag="ls")
            nc.vector.tensor_copy(ls, ctx_ps[64:65, :])
            nc.vector.reciprocal_approx_fast(rl[0:1, h, it % 2, :], ls)

        def norm_head(ctx_tiles, ctxs_pair, h, it):
            """Scale head h's ctx rows by 1/l into the bf16 ctxs tile."""
            pair, half = divmod(h, 2)
            if half == 0:
                ctxs_pair[pair] = ctxp.tile([128, IS], BF16, tag="c",
                                            name="ctxs")
            bcs = bcp.tile([64, IS], F32, tag="bc", name="bcs")
            nc.gpsimd.partition_broadcast(bcs, rl[0:1, h, it % 2, :])
            nc.vector.tensor_mul(
                ctxs_pair[pair][64 * half:64 * half + 64, :],
                ctx_tiles[h][0:64, :], bcs)

        def attention_head(pair, half, it, fill, fbase, fstep):
            h = 2 * pair + half
            hb = 64 * half
            qs = qt[pair][hb:hb + 64, :]
            ks = kt[pair][hb:hb + 64, :]
            njc = 4 * it + 4
            ctx_ps = ps.tile([128, IS], F32, tag="ctx", name=f"ctx{h}")
            pend = None
            for gi, q0 in enumerate(range(0, njc, 2)):
                quad = ps.tile([128, 2, IS], F32, tag="quad", name="quad")
                exps = expp.tile([128, 2, IS], BF16, tag="e", name="exps")
                cmin = max(0, 128 * (q0 - 4 * it))
                recs = []
                for qi in range(2):
                    jc = q0 + qi
                    c0 = max(0, 128 * (jc - 4 * it))
                    diag = jc >= 4 * it
                    nc.tensor.matmul(
                        quad[:, qi, c0:IS],
                        ks[:, 128 * jc:128 * (jc + 1)],
                        qs[:, it * IS + c0:(it + 1) * IS],
                        start=True, stop=not diag)
                    recs.append((qi, jc, c0))
                if q0 >= 4 * it:
                    # one skewed-AP bias matmul adds the -30k triangle into
                    # both diagonal chunks: cols [cmin:cmin+128] for qi=0,
                    # [cmin+128:cmin+256] for qi=1
                    skew = bass.AP(tensor=quad.tensor, offset=cmin,
                                   ap=[[2 * IS, 128], [IS + 128, 2], [1, 128]])
                    nc.tensor.matmul(skew, ident_sb, tneg2_sb,
                                     start=False, stop=True,
                                     skip_group_check=True)
                nc.scalar.activation(
                    exps[:, 0:2, cmin:IS], quad[:, 0:2, cmin:IS],
                    mybir.ActivationFunctionType.Exp, scale=0.125)
                if pend is not None:
                    for qi, jc, c0 in pend[1]:
                        nc.tensor.matmul(
                            ctx_ps[0:65, c0:IS],
                            v4[:, jc, h, :],
                            pend[0][:, qi, c0:IS],
                            start=(jc == 0), stop=(jc == njc - 1))
                pend = (exps, recs)
                fill.pop(fbase + (gi + 0.5) * fstep)
            fill.pop(fbase + 4 * groups * fstep * 0.25)
            for qi, jc, c0 in pend[1]:
                nc.tensor.matmul(
                    ctx_ps[0:65, c0:IS],
                    v4[:, jc, h, :],
                    pend[0][:, qi, c0:IS],
                    start=(jc == 0), stop=(jc == njc - 1))
            recip_l(ctx_ps, h, it)
            return ctx_ps

        # ========== prologue: slice-0 pair-0 projections + V ==========
        pro = Fill()
        st0 = {}
        qk_items(0, "q", 0, st0, pro)
        qk_items(0, "k", 0, st0, pro)
        perm_item(0, "q", 0, st0, pro)
        v_items(0, 0, st0, pro)
        perm_item(0, "k", 0, st0, pro)
        v_items(0, 1, st0, pro)
        pro.drain()

        # ================= stages =================
        HEADS = [(0, 0), (0, 1), (1, 0), (1, 1)]
        done_ctxs = {}
        prev = None            # (ctx_tiles, ctxs_pair, it) of stage it-1
        for it in range(NIS):
            if it + 2 < NIS:
                xt_dma(it + 2)
            fill = Fill()
            pair1_marker = None
            if it == 0:
                # slice-0 pair-1 projections must land before heads 2-3
                qk_items(0, "q", 1, st0, fill)
                qk_items(0, "k", 1, st0, fill)
                perm_item(0, "q", 1, st0, fill)
                perm_item(0, "k", 1, st0, fill)
                pair1_marker = fill.add_marker()
            if it + 1 < NIS:
                proj_items(it + 1, fill)
            groups = 2 * it + 2
            ctx_tiles = {}
            ctxs_pair = [None, None]
            for h, (pair, half) in enumerate(HEADS):
                if h == 0 and prev is not None:
                    norm_head(prev[0], prev[1], 3, prev[2])
                    done_ctxs[prev[2]] = prev[1]
                    if it == 1:
                        outproj_items(done_ctxs[0], 0, fill)
                    elif it == 3:
                        outproj_items(done_ctxs[1], 1, fill)
                        outproj_items(done_ctxs[2], 2, fill)
                if h > 0:
                    norm_head(ctx_tiles, ctxs_pair, h - 1, it)
                if h == 2 and pair1_marker is not None:
                    fill.pop_marker(pair1_marker)
                fbase = h / 4.0
                fstep = 1.0 / (4.0 * groups)
                ctx_tiles[h] = attention_head(pair, half, it, fill,
                                              fbase, fstep)
            fill.drain()
            prev = (ctx_tiles, ctxs_pair, it)

        # ================= epilogue =================
        norm_head(prev[0], prev[1], 3, prev[2])
        outproj_epilogue(prev[1], prev[2])

    nc.compile()
    return nc


def _host_tables():
    inv_freq = 1.0 / (THETA ** (np.arange(0, HD, 2, dtype=np.float64) / HD))
    pos = np.arange(S, dtype=np.float64)
    ang = pos[None, :] * inv_freq[:, None]          # [32, S]
    cos32 = np.cos(ang).astype(np.float32)
    sin32 = np.sin(ang).astype(np.float32)
    cdup = np.concatenate([cos32, cos32, cos32, cos32], axis=0)  # [128, S]
    s_signed = np.concatenate([-sin32, sin32, -sin32, sin32], axis=0)
    p64 = np.zeros((128, 128), dtype=np.float32)
    for m in range(128):
        blk = m - (m % 64)
        d = m % 64
        p64[blk + ((d + 32) % 64), m] = 1.0
    ident = np.eye(128, dtype=np.float32)
    # tneg[j, c] = NEG where j > c (strictly below the block diagonal)
    tneg = np.where(np.arange(128)[:, None] > np.arange(128)[None, :],
                    NEG, 0.0).astype(np.float32)
    tneg2 = np.concatenate([tneg, tneg], axis=1)
    return cdup, s_signed, p64, ident, tneg2


_NC_CACHE = {}


def make_in_maps(x, W_q, W_k, W_v, W_o):
    cdup, sdup, p64, ident, tneg2 = _host_tables()
    ones = np.ones((128, 65), dtype=np.float32)

    def b16(a):
        return np.ascontiguousarray(a.astype(BFNP))

    def wshuf(w):  # [1024, 256] -> [128, 8, 256]
        return b16(w.reshape(8, 128, 256).transpose(1, 0, 2))

    cdup, sdup, p64, ident, tneg2, ones = (b16(a) for a in
                                           (cdup, sdup, p64, ident, tneg2,
                                            ones))
    in_maps = []
    for c in range(NCORES):
        b, g = divmod(c, 4)
        cols = slice(256 * g, 256 * (g + 1))
        # xtr[it, p, ch, i] = x[b][512it+i, 128ch+p]
        xtr = b16(x[b].reshape(NIS, IS, 8, 128).transpose(0, 3, 2, 1))
        in_maps.append({
            "xT": xtr,
            "wq": wshuf(W_q[:, cols]),
            "wk": wshuf(W_k[:, cols]),
            "wv": wshuf(W_v[:, cols]),
            "wo": b16(W_o[cols, :].reshape(2, 128, 1024).transpose(1, 0, 2)),
            "cdup": cdup, "sdup": sdup, "p64": p64, "ident": ident,
            "tneg2": tneg2, "onesc": ones,
        })
    return in_maps


def kernel(x, W_q, W_k, W_v, W_o):
    x = np.ascontiguousarray(x, dtype=np.float32)
    W_q = np.ascontiguousarray(W_q, dtype=np.float32)
    W_k = np.ascontiguousarray(W_k, dtype=np.float32)
    W_v = np.ascontiguousarray(W_v, dtype=np.float32)
    W_o = np.ascontiguousarray(W_o, dtype=np.float32)

    if "nc" not in _NC_CACHE:
        _NC_CACHE["nc"] = build_kernel()
    nc = _NC_CACHE["nc"]

    in_maps = make_in_maps(x, W_q, W_k, W_v, W_o)
    res = run_bass_kernel_spmd(nc, in_maps, list(range(NCORES)))
    outs = [res.results[c]["out"].astype(np.float32) for c in range(NCORES)]
    full = np.empty((B, S, D_OUT), dtype=np.float32)
    for b in range(B):
        full[b] = outs[4 * b] + outs[4 * b + 1] + outs[4 * b + 2] + outs[4 * b + 3]
    return full
